# revision 6
# baseline (speedup 1.0000x reference)
"""CCA kernel v2: 8 axon trn2 cores, matmul-based resizes + bf16 matmuls.

Sharding: 8 shards = (batch b in 0..3) x (row-half in 0..1).
Resizes are expressed as constant interpolation-matrix matmuls (PE-friendly,
no gathers). Heavy einsums run in bf16 (fp32 softmax/accumulate elsewhere).
"""

import os
import numpy as np
import jax

os.makedirs("/root/.cache/jax_neuron", exist_ok=True)
jax.config.update("jax_compilation_cache_dir", "/root/.cache/jax_neuron")
jax.config.update("jax_persistent_cache_min_entry_size_bytes", 0)
jax.config.update("jax_persistent_cache_min_compile_time_secs", 0.0)

import jax.numpy as jnp
from functools import partial

B, C, H, W = 4, 256, 256, 256
Ca, Cq = 64, C // 8
h = w = 127  # conv output spatial


def _interp_matrix(n_in, n_out):
    # bilinear align_corners: out[o] = sum_i M[o,i] * in[i]
    ys = np.linspace(0.0, n_in - 1.0, n_out)
    y0 = np.floor(ys).astype(np.int64)
    y1 = np.minimum(y0 + 1, n_in - 1)
    wy = (ys - y0).astype(np.float32)
    M = np.zeros((n_out, n_in), np.float32)
    M[np.arange(n_out), y0] += 1.0 - wy
    M[np.arange(n_out), y1] += wy
    return M

# downsample 256 -> 127 (for attention_map), upsample 127 -> 256 (for output)
_RD = _interp_matrix(H, h)            # [127, 256]
_RU = _interp_matrix(h, H)            # [256, 127]


@partial(jax.jit, static_argnames=("i0", "y0"), donate_argnums=())
def _shard_fn(x, am, w_down, wq, bq, wk, bk, wv, bv, gamma, *, i0, y0):
    f32, bf16 = jnp.float32, jnp.bfloat16
    x = x.astype(f32)

    # depthwise conv k=4 s=2 VALID as 16 shifted taps (bf16 elementwise)
    xb = x.astype(bf16)
    wd = w_down.astype(bf16)
    xd = jnp.zeros((C, h, w), bf16)
    for kh in range(4):
        for kw in range(4):
            tap = xb[:, kh:kh + 2 * h:2, kw:kw + 2 * w:2]
            xd = xd + tap * wd[:, 0, kh, kw][:, None, None]

    # 1x1 convs (bf16 matmuls, fp32 bias add)
    q = (jnp.einsum('chw,oc->ohw', xd, wq.astype(bf16),
                    preferred_element_type=f32) + bq[:, None, None])
    k = (jnp.einsum('chw,oc->ohw', xd, wk.astype(bf16),
                    preferred_element_type=f32) + bk[:, None, None])
    v = (jnp.einsum('chw,oc->ohw', xd, wv.astype(bf16),
                    preferred_element_type=f32) + bv[:, None, None])

    ni = 64  # rows per slab
    qh = q[:, i0:i0 + ni, :].astype(bf16)
    kb = k.astype(bf16)
    kh_rows = kb[:, i0:i0 + ni, :]

    diag = jnp.eye(h, dtype=bool)[i0:i0 + ni][:, None, :]  # [ni,1,h]

    eH = jnp.einsum('cij,clj->ijl', qh, kb, preferred_element_type=f32)
    eH = jnp.where(diag, -jnp.inf, eH)
    eW = jnp.einsum('cij,cim->ijm', qh, kh_rows, preferred_element_type=f32)
    concate = jax.nn.softmax(jnp.concatenate([eH, eW], axis=2), axis=2)

    # attention_map downsample via interpolation matmuls: [Ca,256,256]->[Ca,h,w]
    rd = jnp.asarray(_RD)
    a = jnp.einsum('iI,cIJ,jJ->cij', rd.astype(bf16), am.astype(bf16),
                   rd.astype(bf16), preferred_element_type=f32)
    ab = a.astype(bf16)
    ah = ab[:, i0:i0 + ni, :]
    aH = jnp.einsum('cij,clj->ijl', ah, ab, preferred_element_type=f32)
    aH = jnp.where(diag, -jnp.inf, aH)
    aW = jnp.einsum('cij,cim->ijm', ah, ah, preferred_element_type=f32)
    concate_a = jax.nn.softmax(jnp.concatenate([aH, aW], axis=2), axis=2)

    S = jax.nn.softmax(concate * concate_a, axis=2)
    attH = S[..., :h].astype(bf16)
    attW = S[..., h:].astype(bf16)

    vb = v.astype(bf16)
    outH = jnp.einsum('ijl,clj->cij', attH, vb, preferred_element_type=f32)
    outW = jnp.einsum('ijm,cim->cij', attW, vb[:, i0:i0 + ni, :],
                      preferred_element_type=f32)

    # bilinear upsample of this 128-row output half via interpolation matmuls
    ru_rows = jnp.asarray(_RU[y0:y0 + 128, i0:i0 + ni])   # [128, ni]
    ru_cols = jnp.asarray(_RU)                            # [256, 127]
    up = jnp.einsum('Yi,cij,Xj->cYX', ru_rows.astype(bf16),
                    (outH + outW).astype(bf16), ru_cols.astype(bf16),
                    preferred_element_type=f32)
    return gamma[0] * up + x[:, y0:y0 + 128, :]


def kernel(x, attention_map, w_down, wq, bq, wk, bk, wv, bv, gamma):
    params = (np.asarray(w_down, np.float32), np.asarray(wq, np.float32),
              np.asarray(bq, np.float32), np.asarray(wk, np.float32),
              np.asarray(bk, np.float32), np.asarray(wv, np.float32),
              np.asarray(bv, np.float32), np.asarray(gamma, np.float32))
    x = np.asarray(x, np.float32)
    attention_map = np.asarray(attention_map, np.float32)

    futs = {}
    cpu = jax.devices("cpu")[0]

    def _device_work(res):
        try:
            import sys, time as _t
            _t0 = _t.time()
            def _lap(tag):
                print(f"[kernel] {tag}: {_t.time() - _t0:.2f}s", file=sys.stderr, flush=True)
            import threading as _th
            devs8 = jax.devices()[:8]
            _lap("devices")
            assert len(devs8) == 8
            # AOT-compile both static variants concurrently (neuronx-cc runs
            # as subprocesses, so these threads overlap).
            arg_structs = [jax.ShapeDtypeStruct(p.shape, p.dtype) for p in params]
            xs = jax.ShapeDtypeStruct((C, H, W), np.float32)
            as_ = jax.ShapeDtypeStruct((Ca, H, W), np.float32)

            def compile_variant(i0, y0, slot):
                res[("exe", slot)] = _shard_fn.lower(
                    xs, as_, *arg_structs, i0=i0, y0=y0).compile()

            ts = [_th.Thread(target=compile_variant, args=(0, 0, 0), daemon=True),
                  _th.Thread(target=compile_variant, args=(63, 128, 1), daemon=True)]
            for t in ts:
                t.start()
            for t in ts:
                t.join()
            _lap("compile")
            dparams = [tuple(jax.device_put(p, d) for p in params) for d in devs8]
            for s in range(8):
                b, half = s // 2, s % 2
                i0, y0 = (0, 0) if half == 0 else (63, 128)
                d = devs8[s]
                xb = jax.device_put(x[b], d)
                ab = jax.device_put(attention_map[b], d)
                futs[s] = _shard_fn(xb, ab, *dparams[s], i0=i0, y0=y0)
            _lap("dispatch")
            for s in range(8):
                res[s] = np.asarray(futs[s])
                _lap(f"fetch{s}")
            res["ok"] = True
        except Exception:
            res["ok"] = False

    import threading
    res = {}
    t = threading.Thread(target=_device_work, args=(res,), daemon=True)
    t.start()
    t.join(float(os.environ.get("CCA_DEVICE_TIMEOUT_S", "1800")))
    futs = {s: res[s] for s in range(8)} if res.get("ok") else {}
    out = np.empty((B, C, H, W), np.float32)
    for s in range(8):
        b, half = s // 2, s % 2
        i0, y0 = (0, 0) if half == 0 else (63, 128)
        try:
            res = np.asarray(futs[s])
        except Exception:
            with jax.default_device(cpu):
                res = np.asarray(_shard_fn(
                    jax.device_put(x[b], cpu), jax.device_put(attention_map[b], cpu),
                    *[jax.device_put(p, cpu) for p in params], i0=i0, y0=y0))
        out[b, :, half * 128:(half + 1) * 128, :] = res
    return out


# revision 7
# speedup vs baseline: 2.4938x; 2.4938x over previous
"""CCA (criss-cross attention) on axon-tunneled trn2 NeuronCores.

Strategy: the workload is transfer-bound over the axon tunnel (device compute
is ~ms; the tunnel moves ~65 MB/s), so the sharding minimizes wire bytes:
one full batch element per core (4 cores busy, B=4) instead of 8 half-image
shards — halving input traffic, since criss-cross column attention needs all
key/value rows and a row-half shard would still need the full image. Inputs
and outputs cross the wire in bf16 (fp32 accumulation on device; final output
upcast on host). Resizes are constant interpolation-matrix matmuls (PE-
friendly, no gathers — also what makes neuronx-cc compile fast).

A persistent jax compilation cache at /root/.cache/jax_neuron makes fresh-
process runs skip neuronx-cc entirely. Device work runs under a watchdog
thread with a per-shard host-CPU fallback, so a device failure degrades to
slow-but-correct.
"""

import os
import numpy as np
import jax

os.makedirs("/root/.cache/jax_neuron", exist_ok=True)
jax.config.update("jax_compilation_cache_dir", "/root/.cache/jax_neuron")
jax.config.update("jax_persistent_cache_min_entry_size_bytes", 0)
jax.config.update("jax_persistent_cache_min_compile_time_secs", 0.0)

import jax.numpy as jnp
import ml_dtypes

B, C, H, W = 4, 256, 256, 256
Ca, Cq = 64, C // 8
h = w = 127  # conv output spatial


def _interp_matrix(n_in, n_out):
    # bilinear align_corners: out[o] = sum_i M[o,i] * in[i]
    ys = np.linspace(0.0, n_in - 1.0, n_out)
    y0 = np.floor(ys).astype(np.int64)
    y1 = np.minimum(y0 + 1, n_in - 1)
    wy = (ys - y0).astype(np.float32)
    M = np.zeros((n_out, n_in), np.float32)
    M[np.arange(n_out), y0] += 1.0 - wy
    M[np.arange(n_out), y1] += wy
    return M

# downsample 256 -> 127 (for attention_map), upsample 127 -> 256 (for output)
_RD = _interp_matrix(H, h)            # [127, 256]
_RU = _interp_matrix(h, H)            # [256, 127]


@jax.jit
def _shard_fn(x, am, w_down, wq, bq, wk, bk, wv, bv, gamma):
    # x: [C,H,W] bf16 one batch element; am: [Ca,H,W] bf16
    f32, bf16 = jnp.float32, jnp.bfloat16

    # depthwise conv k=4 s=2 VALID as 16 shifted taps (bf16 elementwise)
    wd = w_down.astype(bf16)
    xd = jnp.zeros((C, h, w), bf16)
    for kh in range(4):
        for kw in range(4):
            tap = x[:, kh:kh + 2 * h:2, kw:kw + 2 * w:2]
            xd = xd + tap * wd[:, 0, kh, kw][:, None, None]

    # 1x1 convs (bf16 matmuls, fp32 accumulate + bias)
    q = (jnp.einsum('chw,oc->ohw', xd, wq.astype(bf16),
                    preferred_element_type=f32) + bq[:, None, None])
    k = (jnp.einsum('chw,oc->ohw', xd, wk.astype(bf16),
                    preferred_element_type=f32) + bk[:, None, None])
    v = (jnp.einsum('chw,oc->ohw', xd, wv.astype(bf16),
                    preferred_element_type=f32) + bv[:, None, None])

    qb, kb = q.astype(bf16), k.astype(bf16)
    diag = jnp.eye(h, dtype=bool)[:, None, :]          # [h,1,h]

    eH = jnp.einsum('cij,clj->ijl', qb, kb, preferred_element_type=f32)
    eH = jnp.where(diag, -jnp.inf, eH)
    eW = jnp.einsum('cij,cim->ijm', qb, kb, preferred_element_type=f32)
    concate = jax.nn.softmax(jnp.concatenate([eH, eW], axis=2), axis=2)

    # attention_map downsample via interpolation matmuls: [Ca,256,256]->[Ca,h,w]
    rd = jnp.asarray(_RD).astype(bf16)
    a = jnp.einsum('iI,cIJ,jJ->cij', rd, am, rd, preferred_element_type=f32)
    ab = a.astype(bf16)
    aH = jnp.einsum('cij,clj->ijl', ab, ab, preferred_element_type=f32)
    aH = jnp.where(diag, -jnp.inf, aH)
    aW = jnp.einsum('cij,cim->ijm', ab, ab, preferred_element_type=f32)
    concate_a = jax.nn.softmax(jnp.concatenate([aH, aW], axis=2), axis=2)

    S = jax.nn.softmax(concate * concate_a, axis=2)
    attH = S[..., :h].astype(bf16)
    attW = S[..., h:].astype(bf16)

    vb = v.astype(bf16)
    outH = jnp.einsum('ijl,clj->cij', attH, vb, preferred_element_type=f32)
    outW = jnp.einsum('ijm,cim->cij', attW, vb, preferred_element_type=f32)

    # bilinear upsample 127->256 via interpolation matmuls
    ru = jnp.asarray(_RU).astype(bf16)
    up = jnp.einsum('Yi,cij,Xj->cYX', ru, (outH + outW).astype(bf16), ru,
                    preferred_element_type=f32)
    out = gamma[0] * up + x.astype(f32)
    return out.astype(bf16)


def kernel(x, attention_map, w_down, wq, bq, wk, bk, wv, bv, gamma):
    params = (np.asarray(w_down, np.float32), np.asarray(wq, np.float32),
              np.asarray(bq, np.float32), np.asarray(wk, np.float32),
              np.asarray(bk, np.float32), np.asarray(wv, np.float32),
              np.asarray(bv, np.float32), np.asarray(gamma, np.float32))
    xb16 = np.asarray(x).astype(ml_dtypes.bfloat16)
    ab16 = np.asarray(attention_map).astype(ml_dtypes.bfloat16)

    def _device_work(res):
        try:
            import sys, time as _t
            _t0 = _t.time()
            def _lap(tag):
                print(f"[kernel] {tag}: {_t.time() - _t0:.2f}s",
                      file=sys.stderr, flush=True)
            devs = jax.devices()[:B]
            futs = {}
            for b in range(B):
                d = devs[b]
                dp = tuple(jax.device_put(p, d) for p in params)
                futs[b] = _shard_fn(jax.device_put(xb16[b], d),
                                    jax.device_put(ab16[b], d), *dp)
            _lap("dispatch")
            for b in range(B):
                try:
                    futs[b].copy_to_host_async()
                except Exception:
                    pass
            for b in range(B):
                res[b] = np.asarray(futs[b])
                _lap(f"fetch{b}")
            res["ok"] = True
        except Exception as e:
            import sys
            print(f"[kernel] device path failed: {e!r}", file=sys.stderr)
            res["ok"] = False

    import threading
    res = {}
    t = threading.Thread(target=_device_work, args=(res,), daemon=True)
    t.start()
    t.join(float(os.environ.get("CCA_DEVICE_TIMEOUT_S", "1800")))

    out = np.empty((B, C, H, W), np.float32)
    cpu = jax.devices("cpu")[0]
    for b in range(B):
        if res.get("ok") and b in res:
            out[b] = np.asarray(res[b], np.float32)
        else:
            with jax.default_device(cpu):
                out[b] = np.asarray(_shard_fn(
                    jax.device_put(jnp.asarray(xb16[b]), cpu),
                    jax.device_put(jnp.asarray(ab16[b]), cpu),
                    *[jax.device_put(p, cpu) for p in params]), np.float32)
    return out
